# revision 17
# baseline (speedup 1.0000x reference)
"""MoNet (GMM graph conv) 3-layer kernel for one TRN2 chip (8 NeuronCores).

Strategy (graph/data parallel, dst-sharded), v2:
  - Nodes are split into 8 contiguous shards of 2500; core c owns all edges
    whose dst lands in its shard (host-side index prep only).
  - Per layer, each core:
      * computes Gaussian mixture weights w[e,k] on device (DVE+ACT, bf16),
      * dma_gather's h[src[e]] rows (bf16, 256B rows) from a replicated
        full-h DRAM table, spread across 4 SWDGE queues (4 Q7 core pairs
        generate descriptors concurrently),
      * aggregates g_k^T[d,n] for all k at once: per 128-edge tile, one
        matmul with the gathered H tile as the STATIONARY operand (lhsT)
        and the 4 w-scaled one-hot dst masks packed along the free dim as
        rhs -> PSUM [128 d, K*128 n], accumulated over the node bin,
      * applies the dense transform aggT[o,n] = sum_k W_k^T @ g_k^T
        (4 accumulated matmuls, no pre-transposes), adds bias as a
        per-partition scalar fused into the PSUM->SBUF ACT copy, then one
        PE transpose back to row-major [n, o] for the DRAM h table,
      * AllGather's the new h shard to every core (bf16).
  - One-hot dst masks are precomputed host-side and kept SBUF-resident.
  - Compute dtype bf16 (fp32 PSUM accumulation); u/tanh math in fp32.
"""

import sys

sys.path.insert(0, "/opt/trn_rl_repo")

import numpy as np
import ml_dtypes

from concourse import bacc, mybir
from concourse import tile
from concourse.bass_utils import run_bass_kernel_spmd
from concourse.library_config import mlp

N_LAYERS = 3
N_NODES = 20000
N_EDGES = 320000
IN_FEATS = 64
D = 128            # padded feature width, = hidden width for all layers
K = 4
N_CORES = 8
SHARD = N_NODES // N_CORES          # 2500
NT = (SHARD + 127) // 128           # 20 node tiles per core (last has 68 rows)
W = 64                              # node sub-bin width for the mask matmul
NB = (SHARD + W - 1) // W           # 40 sub-bins per core
N_QUEUES = 4
BF = mybir.dt.bfloat16
F32 = mybir.dt.float32
I16 = mybir.dt.int16
bf16 = ml_dtypes.bfloat16


def _plan_edges(src, dst):
    """Partition + sort + pad edges. Returns per-core index arrays and the
    shared per-node-tile tile counts T_bins (identical across cores so the
    single SPMD program fits every core)."""
    core_of = dst // SHARD
    src_half = (src % SHARD) >= 1280        # 0: rows in hagA, 1: rows in hagB
    plans = []
    counts = np.zeros((2, N_CORES, NB), dtype=np.int64)
    per_core = []
    for c in range(N_CORES):
        sel = np.nonzero(core_of == c)[0]
        dl = dst[sel] - c * SHARD
        nt = dl // W
        hf = src_half[sel].astype(np.int64)
        order = np.argsort(hf * NB + nt, kind="stable")
        sel, dl, nt, hf = sel[order], dl[order], nt[order], hf[order]
        per_core.append((sel, dl, nt, hf))
        for h in range(2):
            counts[h, c] = np.bincount(nt[hf == h], minlength=NB)
    T_bins2 = np.maximum(1, (counts.max(axis=1) + 127) // 128).astype(np.int64)
    T_tot = int(T_bins2.sum())
    S0 = int(T_bins2[0].sum())
    for c in range(N_CORES):
        sel, dl, nt, hf = per_core[c]
        srcP = np.zeros(T_tot * 128, dtype=np.int64)
        dstlocP = np.full(T_tot * 128, -1.0, dtype=np.float32)
        origP = np.full(T_tot * 128, -1, dtype=np.int64)
        for h in range(2):
            tbase = 0 if h == 0 else S0
            for b in range(NB):
                m = (hf == h) & (nt == b)
                n = int(m.sum())
                lo = tbase * 128
                srcP[lo : lo + n] = src[sel[m]]
                dstlocP[lo : lo + n] = (dl[m] - b * W).astype(np.float32)
                origP[lo : lo + n] = sel[m]
                tbase += int(T_bins2[h, b])
        plans.append((srcP, dstlocP, origP))
    return T_bins2, T_tot, plans


def _wrap_idx(idx_flat):
    """[n] int -> [128, n//16] int16 gather-index layout (16-partition wrap,
    replicated across the 8 Q7 cores)."""
    n = idx_flat.shape[0]
    w = idx_flat.reshape(n // 16, 16).T.astype(np.int16)
    return np.tile(w, (8, 1)).copy()


def _rep(v):
    """Replicate a scalar/vector across 128 partitions as float32."""
    v = np.asarray(v, dtype=np.float32).reshape(-1)
    return np.tile(v, (128, 1)).copy()


def build_program(T_bins2, T_tot):
    S0 = int(T_bins2[0].sum())
    nc = bacc.Bacc("TRN2", target_bir_lowering=False, debug=False,
                   num_devices=N_CORES, num_swdge_queues=N_QUEUES)

    featP_d = nc.dram_tensor("featP", [128, T_tot, D], BF, kind="ExternalInput")
    idxA_d = nc.dram_tensor("idxA", [128, S0 * 8], I16, kind="ExternalInput")
    idxB_d = nc.dram_tensor("idxB", [128, (T_tot - S0) * 8], I16, kind="ExternalInput")
    onehot_d = nc.dram_tensor("onehot", [128, T_tot, W], BF, kind="ExternalInput")
    pseudo_d = nc.dram_tensor("pseudo", [128, T_tot, 2], F32, kind="ExternalInput")
    ident_d = nc.dram_tensor("ident", [128, 128], BF, kind="ExternalInput")
    identf_d = nc.dram_tensor("identf", [128, 128], F32, kind="ExternalInput")
    fcw_d, pw_d, pb_d, mu_d, isg_d, bias_d = [], [], [], [], [], []
    for l in range(3):
        fcw_d.append(nc.dram_tensor(f"fcw{l}", [128, K, D], BF, kind="ExternalInput"))
        pw_d.append(nc.dram_tensor(f"pw{l}", [128, 4], F32, kind="ExternalInput"))
        pb_d.append(nc.dram_tensor(f"pb{l}", [128, 2], F32, kind="ExternalInput"))
        mu_d.append(nc.dram_tensor(f"mu{l}", [128, 2 * K], F32, kind="ExternalInput"))
        isg_d.append(nc.dram_tensor(f"isg{l}", [128, 2 * K], F32, kind="ExternalInput"))
        bias_d.append(nc.dram_tensor(f"bias{l}", [128, 1], F32, kind="ExternalInput"))
    out_d = nc.dram_tensor("out", [SHARD, D], F32, kind="ExternalOutput")

    AF = mybir.ActivationFunctionType
    OP = mybir.AluOpType

    with tile.TileContext(nc) as tc:
        with (
            tc.tile_pool(name="const", bufs=1) as cpool,
            tc.tile_pool(name="wrk", bufs=2) as wpool,
            tc.tile_pool(name="hbin", bufs=6) as hpool,
            tc.tile_pool(name="maskp", bufs=8) as mpool,
            tc.tile_pool(name="outp", bufs=3) as opool,
            tc.tile_pool(name="gps", bufs=3, space="PSUM") as gpsum,
            tc.tile_pool(name="tps", bufs=2, space="PSUM") as tpsum,
            tc.tile_pool(name="aps", bufs=2, space="PSUM") as apsum,
            tc.tile_pool(name="dram", bufs=1, space="DRAM") as dram,
        ):
            nc.gpsimd.load_library(mlp)

            idxA_sb = cpool.tile([128, S0 * 8], I16)
            idxB_sb = cpool.tile([128, (T_tot - S0) * 8], I16)
            onehot = cpool.tile([128, T_tot, W], BF)
            pseudo = cpool.tile([128, T_tot, 2], F32)
            ident = cpool.tile([128, 128], BF)
            identf = cpool.tile([128, 128], F32)
            nc.sync.dma_start(idxA_sb[:], idxA_d[:])
            nc.sync.dma_start(idxB_sb[:], idxB_d[:])
            nc.sync.dma_start(onehot[:], onehot_d[:])
            nc.sync.dma_start(pseudo[:], pseudo_d[:])
            nc.sync.dma_start(ident[:], ident_d[:])
            nc.sync.dma_start(identf[:], identf_d[:])
            fcw, pwt, pbt, mut, isgt, biast = [], [], [], [], [], []
            for l in range(3):
                fcw.append(cpool.tile([128, K, D], BF, tag=f"fcw{l}", name=f"fcw{l}"))
                pwt.append(cpool.tile([128, 4], F32, tag=f"pw{l}", name=f"pwt{l}"))
                pbt.append(cpool.tile([128, 2], F32, tag=f"pb{l}", name=f"pbt{l}"))
                mut.append(cpool.tile([128, 2 * K], F32, tag=f"mu{l}", name=f"mut{l}"))
                isgt.append(cpool.tile([128, 2 * K], F32, tag=f"isg{l}", name=f"isgt{l}"))
                biast.append(cpool.tile([128, 1], F32, tag=f"bias{l}", name=f"biast{l}"))
                nc.sync.dma_start(fcw[l][:], fcw_d[l][:])
                nc.sync.dma_start(pwt[l][:], pw_d[l][:])
                nc.sync.dma_start(pbt[l][:], pb_d[l][:])
                nc.sync.dma_start(mut[l][:], mu_d[l][:])
                nc.sync.dma_start(isgt[l][:], isg_d[l][:])
                nc.sync.dma_start(biast[l][:], bias_d[l][:])

            # DRAM bounce buffers for the inter-layer AllGather
            shard_t = [dram.tile([SHARD, D], BF, tag=f"shard{l}", name=f"shard{l}") for l in range(2)]
            hagA_t = [dram.tile([1280 * N_CORES, D], BF, tag=f"hagA{l}", name=f"hagA{l}") for l in range(2)]
            hagB_t = [dram.tile([(SHARD - 1280) * N_CORES, D], BF, tag=f"hagB{l}", name=f"hagB{l}") for l in range(2)]

            for l in range(N_LAYERS):

                # ---- Phase W: mixture weights w[e,k], k-inner bf16 layout ----
                w_all = wpool.tile([128, T_tot, K], BF, tag="w_all")
                u = wpool.tile([128, 2, T_tot], F32, tag="u")
                tmp0 = wpool.tile([128, T_tot], F32, tag="tmp0")
                tmp1 = wpool.tile([128, T_tot], F32, tag="tmp1")
                for d in range(2):
                    # u_d = tanh(p0*pw[0,d] + p1*pw[1,d] + pb[d])
                    nc.vector.tensor_scalar(tmp0[:], pseudo[:, :, 0],
                                            pwt[l][:, d : d + 1], None, OP.mult)
                    nc.vector.tensor_scalar(tmp1[:], pseudo[:, :, 1],
                                            pwt[l][:, 2 + d : 3 + d], None, OP.mult)
                    nc.vector.tensor_tensor(tmp0[:], tmp0[:], tmp1[:], OP.add)
                    nc.scalar.activation(u[:, d, :], tmp0[:], AF.Tanh,
                                         bias=pbt[l][:, d : d + 1])
                for k in range(K):
                    nc.vector.tensor_scalar(tmp0[:], u[:, 0, :],
                                            mut[l][:, 2 * k : 2 * k + 1],
                                            isgt[l][:, 2 * k : 2 * k + 1],
                                            OP.subtract, OP.mult)
                    nc.vector.tensor_scalar(tmp1[:], u[:, 1, :],
                                            mut[l][:, 2 * k + 1 : 2 * k + 2],
                                            isgt[l][:, 2 * k + 1 : 2 * k + 2],
                                            OP.subtract, OP.mult)
                    nc.vector.tensor_tensor(tmp0[:], tmp0[:], tmp0[:], OP.mult)
                    nc.vector.tensor_tensor(tmp1[:], tmp1[:], tmp1[:], OP.mult)
                    nc.vector.tensor_tensor(tmp0[:], tmp0[:], tmp1[:], OP.add)
                    nc.scalar.activation(w_all[:, :, k], tmp0[:], AF.Exp, scale=-0.5)

                # ---- Phase E: per node-tile gather + H-stationary matmul ----
                # gather chunks of CH tiles (dma_gather caps at 1024 idxs);
                # layer 0 reads host-pre-gathered rows contiguously instead.
                CH = 8
                chunks = {}

                def get_chunk(t):
                    if l == 0 or t < S0:
                        seg, base, idx_sb, hsrc = 0, 0, idxA_sb, None
                        seg_end = T_tot if l == 0 else S0
                        if l > 0:
                            hsrc = hagA_t[l - 1][:]
                    else:
                        seg, base, idx_sb = 1, S0, idxB_sb
                        seg_end, hsrc = T_tot, hagB_t[l - 1][:]
                    c = (t - base) // CH
                    key = (seg, c)
                    if key not in chunks:
                        n = min(CH, seg_end - base - c * CH)
                        Hc = hpool.tile([128, CH, D], BF, tag="hbin",
                                        name=f"hb_{l}_{seg}_{c}")
                        if hsrc is None:
                            nc.sync.dma_start(
                                Hc[:, :n, :],
                                featP_d[:, base + c * CH : base + c * CH + n, :])
                        else:
                            nc.gpsimd.dma_gather(
                                Hc[:, :n, :], hsrc,
                                idx_sb[:, c * CH * 8 : (c * CH + n) * 8],
                                num_idxs=n * 128, num_idxs_reg=n * 128, elem_size=D,
                                queue_num=c % N_QUEUES,
                            )
                        chunks[key] = Hc
                    return chunks[key]

                def chunk_slot(t):
                    return t % CH if (l == 0 or t < S0) else (t - S0) % CH

                tb0 = np.concatenate([[0], np.cumsum(T_bins2[0])]).astype(int)
                tb1 = np.concatenate([[0], np.cumsum(T_bins2[1])]).astype(int)
                for b in range(NT):
                    # two 64-node sub-bins reassemble one 128-node output tile
                    gsb = opool.tile([128, K, 128], BF, tag="gsb")
                    for half in range(2):
                        sb = 2 * b + half
                        if sb >= NB:
                            continue
                        tlist = (list(range(tb0[sb], tb0[sb + 1])) +
                                 list(range(S0 + tb1[sb], S0 + tb1[sb + 1])))
                        Tn = len(tlist)
                        # w-scaled one-hot masks for the sub-bin, K packed
                        # along the free dim:
                        # wm[e, j, k, n] = w[e, tbase+j, k] * oh[e, tbase+j, n]
                        wm = mpool.tile([128, Tn, K, W], BF, tag="wmask")
                        wm_eng = (nc.gpsimd if (l == 0 and sb % 3 == 2)
                                  else nc.vector)
                        n0 = tb0[sb + 1] - tb0[sb]
                        for (r0, r1, w0) in (
                            (tb0[sb], tb0[sb + 1], 0),
                            (S0 + tb1[sb], S0 + tb1[sb + 1], n0),
                        ):
                            rn = r1 - r0
                            wm_eng.tensor_tensor(
                                wm[:, w0 : w0 + rn, :, :],
                                onehot[:, r0:r1, :].unsqueeze(2)
                                    .broadcast_to([128, rn, K, W]),
                                w_all[:, r0:r1, :].unsqueeze(3)
                                    .broadcast_to([128, rn, K, W]),
                                OP.mult,
                            )
                        gp = gpsum.tile([128, K * W], F32, tag="g")
                        for j, t in enumerate(tlist):
                            Hc = get_chunk(t)
                            nc.tensor.matmul(gp[:], Hc[:, chunk_slot(t), :],
                                             wm[:, j, :, :],
                                             start=(j == 0), stop=(j == Tn - 1))
                        # gT[d, (k,n)] -> its half of the paired SBUF buffer
                        nc.scalar.activation(
                            gsb[:, :, half * W : (half + 1) * W],
                            gp[:].rearrange("p (k n) -> p k n", k=K), AF.Copy)
                    # aggT[o,n] = sum_k W_k^T @ g_k^T over the 128-node pair
                    aggp = apsum.tile([128, 128], F32, tag="agg")
                    for k in range(K):
                        nc.tensor.matmul(aggp[:], fcw[l][:, k, :], gsb[:, k, :],
                                         start=(k == 0), stop=(k == 3))
                    rows = min(128, SHARD - b * 128)
                    if l < N_LAYERS - 1:
                        # bias is a per-partition scalar in [o, n] layout
                        ht = opool.tile([128, 128], F32, tag="hout")
                        nc.scalar.activation(ht[:], aggp[:], AF.Identity,
                                             bias=biast[l][:, 0:1])
                        trp = tpsum.tile([128, 128], F32, tag="tr")
                        nc.tensor.transpose(trp[:], ht[:], identf[:])
                        hrow = opool.tile([128, 128], BF, tag="hrow")
                        nc.scalar.activation(hrow[:], trp[:], AF.Copy)
                        nc.sync.dma_start(
                            shard_t[l][b * 128 : b * 128 + rows, :], hrow[:rows, :])
                    else:
                        hf = opool.tile([128, 128], F32, tag="hfin")
                        nc.scalar.activation(hf[:], aggp[:], AF.Identity,
                                             bias=biast[l][:, 0:1])
                        trf = tpsum.tile([128, 128], F32, tag="tr")
                        nc.tensor.transpose(trf[:], hf[:], identf[:])
                        hfr = opool.tile([128, 128], F32, tag="hfr")
                        nc.scalar.activation(hfr[:], trf[:], AF.Copy)
                        nc.sync.dma_start(
                            out_d[b * 128 : b * 128 + rows, :], hfr[:rows, :])
                    if l < 2 and b == 9:
                        # first half-shard AllGather overlaps bins 10-19
                        nc.gpsimd.collective_compute(
                            "AllGather", OP.bypass,
                            replica_groups=[list(range(N_CORES))],
                            ins=[shard_t[l][0:1280, :]],
                            outs=[hagA_t[l].opt()],
                        )

                if l < 2:
                    nc.gpsimd.collective_compute(
                        "AllGather", OP.bypass,
                        replica_groups=[list(range(N_CORES))],
                        ins=[shard_t[l][1280:SHARD, :]],
                        outs=[hagB_t[l].opt()],
                    )
    nc.compile()
    return nc


def _host_inputs(inputs, T_bins, T_tot, plans):
    """Build the 8 per-core input maps."""
    feats = np.zeros((N_NODES, D), dtype=np.float32)
    feats[:, :IN_FEATS] = inputs["features"]
    feat_bf = feats.astype(bf16)
    ident = np.eye(128, dtype=np.float32).astype(bf16)

    common = {"ident": ident, "identf": np.eye(128, dtype=np.float32)}
    for l in range(3):
        fc = np.asarray(inputs[f"fc_w{l}"], dtype=np.float32)   # [din, K*128]
        fcp = np.zeros((D, K * D), dtype=np.float32)
        fcp[: fc.shape[0], :] = fc
        fcw = fcp.reshape(D, K, D).astype(bf16)                  # [j, k, o]
        common[f"fcw{l}"] = fcw
        pw = np.asarray(inputs[f"pw{l}"], dtype=np.float32)      # [2,2]
        common[f"pw{l}"] = _rep([pw[0, 0], pw[0, 1], pw[1, 0], pw[1, 1]])
        common[f"pb{l}"] = _rep(inputs[f"pb{l}"])
        common[f"mu{l}"] = _rep(np.asarray(inputs[f"mu{l}"]).reshape(-1))
        common[f"isg{l}"] = _rep(np.asarray(inputs[f"inv_sigma{l}"]).reshape(-1))
        common[f"bias{l}"] = (np.asarray(inputs[f"bias{l}"], dtype=np.float32)
                              .reshape(128, 1).copy())

    pseudo = np.asarray(inputs["pseudo"], dtype=np.float32)
    iotaW = np.arange(W, dtype=np.float32)
    in_maps = []
    for c in range(N_CORES):
        srcP, dstlocP, origP = plans[c]
        m = dict(common)
        S0 = int(T_bins[0].sum())
        c_of = srcP // SHARD
        loc = srcP % SHARD
        srcR = np.where(loc < 1280, c_of * 1280 + loc,
                        c_of * (SHARD - 1280) + (loc - 1280))
        m["idxA"] = _wrap_idx(srcR[: S0 * 128])
        m["idxB"] = _wrap_idx(srcR[S0 * 128 :])
        # layer-0 source rows pre-gathered into edge order (input sharding)
        m["featP"] = (feat_bf[srcP].reshape(T_tot, 128, D)
                      .transpose(1, 0, 2).copy())
        dstloc = dstlocP.reshape(T_tot, 128).T          # [128 e, T_tot]
        m["onehot"] = (dstloc[:, :, None] == iotaW[None, None, :]).astype(bf16)
        ps = np.zeros((T_tot * 128, 2), dtype=np.float32)
        valid = origP >= 0
        ps[valid] = pseudo[origP[valid]]
        m["pseudo"] = ps.reshape(T_tot, 128, 2).transpose(1, 0, 2).copy()
        in_maps.append(m)
    return in_maps


_CACHE = {}


def _get_compiled(src, dst):
    key = (src.tobytes(), dst.tobytes())
    h = hash(key)
    if h not in _CACHE:
        T_bins, T_tot, plans = _plan_edges(np.asarray(src, dtype=np.int64),
                                           np.asarray(dst, dtype=np.int64))
        nc = build_program(T_bins, T_tot)
        _CACHE[h] = (nc, T_bins, T_tot, plans)
    return _CACHE[h]


def run(inputs, trace=False, **kwargs):
    nc, T_bins, T_tot, plans = _get_compiled(
        np.asarray(inputs["src"]), np.asarray(inputs["dst"]))
    in_maps = _host_inputs(inputs, T_bins, T_tot, plans)
    res = run_bass_kernel_spmd(nc, in_maps, core_ids=list(range(N_CORES)),
                               trace=trace, **kwargs)
    out = np.concatenate([res.results[c]["out"] for c in range(N_CORES)], axis=0)
    return out.astype(np.float32), res


def kernel(**inputs):
    out, _ = run(inputs)
    return out
